# revision 1
# baseline (speedup 1.0000x reference)
import numpy as np

# GaussianUpsampler: B=32, T=512, D=384, outlen ~ max(sum(durations, axis=1))
# Sharding: data-parallel over batch B across 8 NeuronCores (4 batches/core),
# each core computes its Gaussian weight tile [b_loc, outlen, T] and the local
# matmul against feats [b_loc, T, D]. No cross-device communication.

R2PI = float(np.sqrt(2.0 * np.pi))
N_CORES = 8


def _upsample_np(feats, rng, durations, outlen):
    d = durations.astype(np.float32)
    c = d / 2.0 + np.cumsum(d, axis=-1)
    r = rng.astype(np.float32) + 1e-6
    t = np.arange(outlen, dtype=np.float32)
    z = (t[None, :, None] - c[:, None, :]) / r[:, None, :]
    w = np.exp(-0.5 * z * z) / (r[:, None, :] * R2PI) + 1e-6
    w /= w.sum(axis=2, keepdims=True)
    return np.matmul(w, feats.astype(np.float32))


def _upsample_jax_pmap(feats, rng, durations, outlen):
    import jax
    import jax.numpy as jnp

    devs = jax.devices()[:N_CORES]
    B = feats.shape[0]
    b_loc = B // N_CORES

    f_sh = feats.reshape(N_CORES, b_loc, *feats.shape[1:])
    r_sh = rng.reshape(N_CORES, b_loc, *rng.shape[1:])
    d_sh = durations.reshape(N_CORES, b_loc, *durations.shape[1:])

    def local(f, r, du):
        d = du.astype(jnp.float32)
        c = d / 2.0 + jnp.cumsum(d, axis=-1)
        rr = r + 1e-6
        t = jnp.arange(outlen, dtype=jnp.float32)
        z = (t[None, :, None] - c[:, None, :]) / rr[:, None, :]
        w = jnp.exp(-0.5 * z * z) / (rr[:, None, :] * R2PI) + 1e-6
        w = w / jnp.sum(w, axis=2, keepdims=True)
        return jnp.matmul(w, f)

    out = jax.pmap(local, devices=devs)(f_sh, r_sh, d_sh)
    return np.asarray(out).reshape(B, outlen, feats.shape[2])


def kernel(feats, rng, durations, outlen):
    outlen = int(np.asarray(outlen))
    feats = np.asarray(feats, dtype=np.float32)
    rng = np.asarray(rng, dtype=np.float32)
    durations = np.asarray(durations)
    try:
        return _upsample_jax_pmap(feats, rng, durations, outlen)
    except Exception:
        return _upsample_np(feats, rng, durations, outlen)
